# revision 65
# baseline (speedup 1.0000x reference)
"""Bahdanau-style additive attention kernel for Trainium2 (8 NeuronCores).

reference:
    q_h    = relu(query @ w1.T)                      (B, H)
    k_h    = relu(key @ w2.T)                        (B, T, H)
    scores = tanh(q_h[:, None, :] + k_h) @ w_out     (B, T)
    attn   = softmax(scores, axis=1)
    out    = einsum('bt,bth->bh', attn, key)         (B, H)

B=128, T=2048, H=512. Pure data parallel: 16 batch rows per core.

Device strategy (per core), fully row-pipelined:
  pass 1: the k_h matmul runs in fp8e4m3 DoubleRow mode (contraction
  over 256 h per instruction -> 4x bf16 FLOP rate).  The host
  pre-transposes key -> keyT [b, h, t] fp8 so the contraction dim h sits
  on partitions.  tanh(k_h_raw + q_h) is ONE ScalarE activation reading
  PSUM with per-partition bias q_h; the relu is folded in afterwards via
  the exact identity tanh(relu(k) + q) = max(tanh(k + q), tanh(q)) for
  q >= 0, which runs on the DVE in bf16 at 4x rate.  q_h and tanh(q_h)
  are precomputed on the host (tiny).

  scores are produced TRANSPOSED: each matmul takes a 128-wide t-block
  of th as the stationary operand (lhsT, [128g x 128t]) and w_out as the
  single moving column, accumulating scoresT[t] into a per-row
  [128, 16] PSUM tile.  Each scoresT column finishes its 4-subtile g
  accumulation before the next column starts, so each 2KB PSUM region
  hosts one accumulation group at a time.  This layout means:
    - softmax runs per row: exp reads the [128,16] PSUM tile directly,
      a DVE tensor_reduce forms per-partition partial sums, a
      ones[128x128] matmul broadcast-reduces them into a replicated
      total, and the DVE computes 1/Z;
    - 1/Z is folded into the pass-2 PSUM->SBUF output move
      (tensor_scalar_mul), so pass 2 runs on the UNNORMALIZED attn and
      the softmax normalize never sits on the critical path;
    - the result IS the pass-2 layout (t on partitions), so no attn
      transposes, no [GROUP, T] gathers, no stage copies.
  pass 2: out = attn @ key uses the natural-layout bf16 key tile as the
  stationary operand (lhsT) and the attn column [128t, 1] as the moving
  operand; each output column finishes all 16 t-subtiles before the
  next starts.  Row softmax runs at lag 1 and pass 2 at lag 2 behind
  pass 1, so every engine stream stays dense.

  Queue discipline: SP carries keyT fp8 row loads (three rows ahead of
  use, first rows split into t-halves for a fast start) + deferred
  output stores; the Pool swdge queue carries the bf16 natural-key
  prefetches, whose 5-deep tile ring doubles as flow control keeping kb
  transfers from crowding kt out of the shared DMA-engine FIFO; the
  Activation queue issues nothing, so tanh dispatch never sits behind a
  DMA.  q_h/tanh(q_h)/w_out ship as ONE bundled early DMA.  A scratch
  activation pulls the act-table load to t~0 and dummy PE transposes
  pre-ramp the tensor engine's p-state.  Output goes to a
  partition-major [P, BPC, HS] DRAM tensor (64B-contiguous descriptors)
  and is reassembled on the host.  The softmax skips max-subtraction
  (scores are bounded by ||w_out||_1, exp cannot overflow fp32).
"""

import numpy as np
import ml_dtypes

import concourse.bass as bass
from concourse import bacc
import concourse.mybir as mybir
import concourse.tile as tile
from concourse import bass_utils

B, T, H = 128, 2048, 512
NCORES = 8
BPC = B // NCORES          # 16 batch rows per core
P = 128
HS = H // P                # 4 subtiles of the h/g dims
TC = 512                   # psum-bank sized t-chunk (fp32)
NCH = T // TC              # 4 chunks
TP = T // P                # 16 t-subtiles (scoresT / pass2 layout)
OB = 4                     # rows per output-store batch

f32 = mybir.dt.float32
bf16 = mybir.dt.bfloat16
f8 = mybir.dt.float8e4
AF = mybir.ActivationFunctionType
AX = mybir.AxisListType
PM = mybir.MatmulPerfMode

_CACHE = {}


def _build_nc():
    nc = bacc.Bacc(trn_type="TRN2", target_bir_lowering=False)

    keyT_f8 = nc.dram_tensor("keyT_f8", [BPC, H, T], f8, kind="ExternalInput")
    key_bf = nc.dram_tensor("key_bf", [BPC, T, H], bf16, kind="ExternalInput")
    w2T_f8 = nc.dram_tensor("w2T_f8", [H, H], f8, kind="ExternalInput")
    # planes: 0 = q_h^T, 1 = tanh(q_h)^T, 2 = w_out broadcast along b
    qtw = nc.dram_tensor("qtw", [3, H, BPC], f32, kind="ExternalInput")
    ones = nc.dram_tensor("ones", [P, P], f32, kind="ExternalInput")
    # partition-major output: out_raw[p, b, c] = out[b, c*128 + p]
    out = nc.dram_tensor("out", [P, BPC, HS], f32, kind="ExternalOutput")

    with tile.TileContext(nc) as tc:
        with (
            tc.tile_pool(name="const", bufs=1) as cpool,
            tc.tile_pool(name="kt", bufs=6) as kt_pool,
            tc.tile_pool(name="kb", bufs=5) as kb_pool,
            tc.tile_pool(name="th", bufs=16) as th_pool,
            tc.tile_pool(name="sm", bufs=4) as sm_pool,
            tc.tile_pool(name="osb", bufs=2) as out_pool,
            tc.tile_pool(name="ps_kh", bufs=2, space="PSUM") as ps_kh,
            tc.tile_pool(name="ps_st", bufs=2, space="PSUM") as ps_st,
            tc.tile_pool(name="ps_tot", bufs=1, space="PSUM") as ps_tot,
            tc.tile_pool(name="ps_o", bufs=1, space="PSUM") as ps_o,
        ):
            # ---- startup.  All DMA arrivals are ordered by compute need:
            # SP (fast hwdge issue) carries kt(0), w2T, then the tiny
            # per-partition constants, then the kt prefetch stream (ring 6,
            # three rows ahead) so kt never queues behind the 5.8us kb
            # transfers.  Pool carries `ones` (needed last) as a spacer and
            # then the kb stream. ----
            kt_tiles = {}

            def load_kt(b, halves=False, q=None):
                q = q or nc.sync
                kt = kt_pool.tile([P, HS, NCH, TC], f8, tag="kt",
                                  name="kt%d" % b)
                if halves:
                    # early rows load in two t-chunk halves, matching the
                    # cc compute order: the row's first tanh block only
                    # needs t chunks 0-1, so it fires ~1.5us sooner.
                    for c in range(0, NCH, 2):
                        q.dma_start(
                            kt[:, :, c:c + 2, :],
                            keyT_f8.ap()[b, :, c * TC:(c + 2) * TC].rearrange(
                                "(s p) (c t) -> p s c t", p=P, c=2),
                        )
                else:
                    q.dma_start(
                        kt[:],
                        keyT_f8.ap()[b].rearrange(
                            "(s p) (c t) -> p s c t", p=P, c=NCH),
                    )
                kt_tiles[b] = kt

            # a throwaway activation on a scratch tile makes the framework
            # place the (Tanh|Exp) table load at t~0 instead of just before
            # the first real tanh; its input is uninitialized SBUF and its
            # output is never read.
            scratch = cpool.tile([1, 8], f32)
            nc.scalar.activation(scratch[:], scratch[:], AF.Tanh)
            # dummy transposes keep the PE busy from t~0 so the p-state has
            # fully ramped (3us of continuous execution) before the first
            # real k_h matmul; results are garbage and never read.
            scr2 = cpool.tile([P, P], f32)
            nc.vector.memset(scr2[:], 0.0)
            for w in range(20):
                pwarm = ps_kh.tile([P, 2, TC], f32, tag="kh",
                                   name="warm%d" % w)
                nc.tensor.transpose(pwarm[:, 0, 0:P], scr2[:], scr2[:])

            # The bulk streams (w2T, kt) own the SP queue; the tiny
            # per-partition constants ride the otherwise-idle Activation
            # hwdge queue so they all land within the first ~7us (a late
            # tqT gates the DVE maxes, and through the scheduler's PE
            # interleaving + 4-deep wait queues, the whole pipeline).
            w2T_sb = cpool.tile([P, HS, H], f8)        # [p, h_sub, g]
            nc.sync.dma_start(w2T_sb[:], w2T_f8.ap().rearrange("(s p) g -> p s g", p=P))
            # one bundled DMA delivers q_h, tanh(q_h) and w_out together,
            # early and atomically (a straggling tqT gates the DVE maxes
            # and, through scheduler interleaving, the whole pipeline)
            qtw_sb = cpool.tile([P, 3, HS, BPC], f32)  # [g_sub, plane, gs, b]
            nc.sync.dma_start(
                qtw_sb[:], qtw.ap().rearrange("q (s p) b -> p q s b", p=P))
            qhT_sb = qtw_sb[:, 0]                      # [g_sub, gs, b]
            tqT_sb = qtw_sb[:, 1]                      # tanh(q_h), same layout
            load_kt(0, halves=True)
            load_kt(1, halves=True)
            wout_sb = cpool.tile([P, HS], bf16)        # [p, g_sub]
            nc.vector.tensor_copy(wout_sb[:], qtw_sb[:, 2, :, 0])
            ones_sb = cpool.tile([P, P], f32)
            nc.gpsimd.dma_start(ones_sb[:], ones.ap())
            load_kt(2)

            kb_tiles = {}       # b -> kb tile (freed by pass2)
            st_tiles = {}       # b -> scoresT psum tile (freed by exp)
            attn_tiles = {}     # b -> normalized attnT [128, TP] bf16
            sm_chain = {}       # b -> (sums, tot, inv) in flight
            osb_tiles = {}      # batch index -> osb tile
            pending_out = []    # deferred output DMA batches

            def flush_out(keep_last):
                while len(pending_out) > keep_last:
                    b0, osb = pending_out.pop(0)
                    nc.sync.dma_start(
                        out.ap()[:, b0:b0 + OB, :], osb[:])

            def load_kb(b):
                kb = kb_pool.tile([P, TP, H], bf16, tag="kb",
                                  name="kb%d" % b)
                nc.gpsimd.dma_start(
                    kb[:],
                    key_bf.ap()[b].rearrange("(c p) h -> p c h", p=P),
                )
                kb_tiles[b] = kb

            def pass1(b):
                if b + 3 < BPC:
                    load_kt(b + 3)
                kt = kt_tiles.pop(b)
                # prefetch natural-layout rows for pass 2 on the Pool queue.
                # kb(0) is pushed back to row 1 so the early kt loads are
                # not queued behind its 5.8us transfer.
                if b >= 1:
                    load_kb(b - 1)
                if b == BPC - 1:
                    load_kb(b)

                st = ps_st.tile([P, TP], f32, tag="st", name="st%d" % b)
                st_tiles[b] = st
                # phase A: the whole row's k_h -> tanh -> max chain first,
                # so in PE program order nothing DVE-dependent sits between
                # the kh blocks: the ACT stream ping-pongs with kh on the
                # 2-deep PSUM ring and never waits on maxes or scores.
                ths = {}
                for cc in range(NCH // 2):         # pairs of t-chunks
                    for gs in range(HS):
                        ps = ps_kh.tile([P, 2, TC], f32, tag="kh")
                        for ci in range(2):
                            for i in range(2):     # hs pairs (DoubleRow)
                                nc.tensor.matmul(
                                    ps[:, ci, :],
                                    lhsT=w2T_sb[:, 2 * i:2 * i + 2,
                                                gs * P:(gs + 1) * P],
                                    rhs=kt[:, 2 * i:2 * i + 2, cc * 2 + ci, :],
                                    start=(i == 0),
                                    stop=(i == 1),
                                    perf_mode=PM.DoubleRow,
                                )
                        # tanh(k_raw + q); relu folded in via an in-place
                        # max with tanh(q) (exact for q >= 0)
                        th = th_pool.tile([P, 2, TC], bf16, tag="th")
                        nc.scalar.activation(
                            th[:], ps[:], AF.Tanh,
                            bias=qhT_sb[:, gs, b:b + 1],
                        )
                        nc.vector.tensor_scalar_max(
                            th[:], th[:], tqT_sb[:, gs, b:b + 1])
                        ths[cc, gs] = th
                # phase B: scoresT blocks.  th t-columns become the
                # stationary operand, w_out the single moving column.  Each
                # scoresT column completes its 4-subtile accumulation before
                # the next column starts (one accumulation group per 2KB
                # PSUM region at a time).
                for cc in range(NCH // 2):
                    for ci in range(2):
                        for blk in range(HS):
                            c2 = (cc * 2 + ci) * HS + blk
                            for gs in range(HS):
                                nc.tensor.matmul(
                                    st[:, c2:c2 + 1],
                                    lhsT=ths[cc, gs][:, ci,
                                                     blk * P:(blk + 1) * P],
                                    rhs=wout_sb[:, gs:gs + 1],
                                    start=(gs == 0),
                                    stop=(gs == HS - 1),
                                )

            def softmax(b):
                # exp reads the scoresT PSUM tile directly; the per-partition
                # partial sums over the 16 free t-blocks run on the DVE
                # (cheaper than the ACT accumulator read-out).
                at = sm_pool.tile([P, TP], bf16, tag="attn", name="attn%d" % b)
                sums = sm_pool.tile([P, 1], f32, tag="sums", name="sums%d" % b)
                nc.scalar.activation(at[:], st_tiles.pop(b)[:], AF.Exp)
                nc.vector.tensor_reduce(
                    sums[:], at[:], AX.X, mybir.AluOpType.add)
                attn_tiles[b] = at
                # broadcast-reduce: tot[m] = sum_p sums[p] for every m
                tot = ps_tot.tile([P, 1], f32, tag="tot", name="tot%d" % b)
                nc.tensor.matmul(tot[:], lhsT=ones_sb[:], rhs=sums[:],
                                 start=True, stop=True)
                inv = sm_pool.tile([P, 1], f32, tag="inv", name="inv%d" % b)
                nc.vector.reciprocal(inv[:], tot[:])
                sm_chain[b] = inv

            def pass2(b):
                # runs on the UNNORMALIZED attn; 1/Z (replicated on every
                # partition) is folded into the PSUM->SBUF output move, so
                # the softmax normalize never sits on the critical path.
                kb = kb_tiles.pop(b)
                at = attn_tiles.pop(b)
                inv = sm_chain.pop(b)
                if b % OB == 0:
                    osb_tiles[b // OB] = out_pool.tile(
                        [P, OB, HS], f32, tag="osb", name="osb%d" % b)
                osb = osb_tiles[b // OB]
                pso = ps_o.tile([P, HS], f32, tag="o")
                for hc in range(HS):
                    for c2 in range(TP):
                        nc.tensor.matmul(
                            pso[:, hc:hc + 1],
                            lhsT=kb[:, c2, hc * P:(hc + 1) * P],
                            rhs=at[:, c2:c2 + 1],
                            start=(c2 == 0),
                            stop=(c2 == TP - 1),
                        )
                nc.vector.tensor_scalar_mul(osb[:, b % OB, :], pso[:],
                                            inv[:, 0:1])
                if b % OB == OB - 1:
                    pending_out.append((b - OB + 1, osb_tiles.pop(b // OB)))

            # row-level software pipeline: softmax at lag 1 (placed before
            # pass1 so its deps are older than the tanh deps behind it in
            # the ACT queue), pass2 at lag 2, output stores deferred one
            # row further.
            for b in range(BPC):
                if b >= 1:
                    softmax(b - 1)
                flush_out(keep_last=0)
                pass1(b)
                if b >= 2:
                    pass2(b - 2)
            pass2(BPC - 2)
            softmax(BPC - 1)
            pass2(BPC - 1)
            flush_out(keep_last=0)

    nc.compile()
    return nc


def kernel(query, key, w1, w2, w_out):
    query = np.asarray(query, dtype=np.float32)
    key = np.asarray(key, dtype=np.float32)
    w1 = np.asarray(w1, dtype=np.float32)
    w2 = np.asarray(w2, dtype=np.float32)
    w_out = np.asarray(w_out, dtype=np.float32)

    if "nc" not in _CACHE:
        _CACHE["nc"] = _build_nc()
    nc = _CACHE["nc"]

    f8np = ml_dtypes.float8_e4m3
    w2T_f8 = np.ascontiguousarray(w2.T).astype(f8np)
    ones = np.ones((P, P), dtype=np.float32)

    # host-side q_h = relu(query @ w1.T) and tanh(q_h), transposed [H, B]
    qh = np.maximum(query @ w1.T, 0.0).astype(np.float32)
    qhT = np.ascontiguousarray(qh.T)
    tqT = np.ascontiguousarray(np.tanh(qh.T)).astype(np.float32)
    wout_bc = np.broadcast_to(
        w_out.astype(np.float32).reshape(H, 1), (H, BPC))

    in_maps = []
    for c in range(NCORES):
        sl = slice(c * BPC, (c + 1) * BPC)
        key_c = key[sl]
        qtw = np.ascontiguousarray(
            np.stack([qhT[:, sl], tqT[:, sl], wout_bc], axis=0),
            dtype=np.float32)
        in_maps.append({
            "keyT_f8": np.ascontiguousarray(
                key_c.transpose(0, 2, 1)).astype(f8np),
            "key_bf": np.ascontiguousarray(key_c).astype(ml_dtypes.bfloat16),
            "w2T_f8": w2T_f8,
            "qtw": qtw,
            "ones": ones,
        })

    _CACHE["in_maps"] = in_maps
    res = None
    last_exc = None
    for _attempt in range(3):
        try:
            res = bass_utils.run_bass_kernel_spmd(
                nc, in_maps, core_ids=list(range(NCORES)), trace=False)
            break
        except Exception as e:  # transient device wedge: retry
            last_exc = e
            import time as _time
            _time.sleep(2.0)
    if res is None:
        raise last_exc
    # out_raw[p, b, c] = out[b, c*128 + p]  ->  out[b, h]
    outs = []
    for r in res.results:
        raw = np.asarray(r["out"])          # [P, BPC, HS]
        outs.append(raw.transpose(1, 2, 0).reshape(BPC, H))
    out = np.concatenate(outs, axis=0)
    return out.astype(np.float32)
